# revision 19
# baseline (speedup 1.0000x reference)
"""Causal self-attention Trainium2 kernel.

Reference (full): x[B=2,S=2048,D=1024] @ W_qkv + b_qkv -> 16-head causal
attention -> @ W_out + b_out.

Sharding: 8 cores = (batch b in 0..1) x (head-group hg in 0..3, 4 heads of
hd=64 each). Each core computes a partial output projection for its 4 heads
on its batch; the host sums the 4 head-group partials per batch and adds the
(constant) V-bias correction bv @ W_out and b_out.

Device pipeline per core (data path fp16, accumulation fp32 PSUM):
  The 4 local heads form 2 pairs (2p, 2p+1) whose q/k features live on
  partition halves 0..63 / 64..127 of the projected qkt tile.  Score
  matmuls for a pair are emitted back-to-back as K=64 row-tiled matmuls on
  disjoint row groups (tile_position (0,0) / (64,0)), so the PE array runs
  both heads CONCURRENTLY at full 128-row occupancy - 2x score throughput
  vs serial K=64 matmuls, and enough PE-cell activity to keep the HAM
  clock-gate at 8/8.
  Per (q-span, pair, k-chunk): paired score MMs -> one exp on ACT (scale
  1/8) -> triangle mask on diagonal blocks -> two PV matmuls (M=65: head
  dims + a ones column that yields the softmax denominator for free).
  Normalization: reciprocal of the denominator row on DVE, cast bf16, K=1
  bf16 matmul broadcasts 1/d across partitions (1 cyc/row vs 4 for f32),
  multiply on eviction; odd heads staged through SBUF and DMA'd to
  partitions 64..127.
  x-transposes for q-span j are emitted just before span j's work so the
  first attention block starts ~3us in; projections / output projections
  interleave into the ACT-bound attention stream via the tile scheduler.
"""
import numpy as np
from contextlib import ExitStack

import concourse.bacc as bacc
import concourse.tile as tile
from concourse import mybir
from concourse.bass_utils import run_bass_kernel_spmd

F32 = mybir.dt.float32
BF16 = mybir.dt.bfloat16
F16 = mybir.dt.float16

B = 2
S = 2048
D = 1024
HD = 64
HG = 4            # head-groups (cores per batch)
HPG = 4           # heads per group
CL = HPG * HD     # 256 local head cols per core
P = 128
NSC = S // P      # 16 s-chunks
NDC = D // P      # 8 d-chunks
NQJ = S // 512    # 4 q-spans
NKC = S // P      # 16 k-chunks

_CACHED = {}


def _build():
    if "nc" in _CACHED:
        return _CACHED["nc"]
    nc = bacc.Bacc("TRN2", target_bir_lowering=False, debug=False)

    x_d = nc.dram_tensor("x", [S, D], F16, kind="ExternalInput")
    wqk_d = nc.dram_tensor("wqk", [D, 2 * CL], F16, kind="ExternalInput")
    wv_d = nc.dram_tensor("wv", [D, CL], F16, kind="ExternalInput")
    wout_d = nc.dram_tensor("wout", [CL, D], F16, kind="ExternalInput")
    bqk_d = nc.dram_tensor("bqk", [P, 4], F32, kind="ExternalInput")
    tri_d = nc.dram_tensor("tri", [P, P], F16, kind="ExternalInput")
    idn_d = nc.dram_tensor("idn", [P, P], F16, kind="ExternalInput")
    y_d = nc.dram_tensor("y", [S, D], F32, kind="ExternalOutput")

    with tile.TileContext(nc) as tc, ExitStack() as ctx:
        persist = ctx.enter_context(tc.tile_pool(name="persist", bufs=1))
        ptp = ctx.enter_context(tc.tile_pool(name="ptp", bufs=3))
        youtp = ctx.enter_context(tc.tile_pool(name="youtp", bufs=2))
        unp = ctx.enter_context(tc.tile_pool(name="unp", bufs=2))
        rcpf = ctx.enter_context(tc.tile_pool(name="rcpf", bufs=2))
        rcpb = ctx.enter_context(tc.tile_pool(name="rcpb", bufs=2))
        tmpp = ctx.enter_context(tc.tile_pool(name="tmpp", bufs=2))
        ps_sm = ctx.enter_context(tc.tile_pool(name="ps_sm", bufs=2, space="PSUM"))
        ps_st = ctx.enter_context(tc.tile_pool(name="ps_st", bufs=2, space="PSUM"))
        ps_av = ctx.enter_context(tc.tile_pool(name="ps_av", bufs=2, space="PSUM"))

        # ---- persistent tiles ----
        qkt_sb = persist.tile([P, 4, S], F16, name="qkt_sb")        # 16KB/part
        v_sb = persist.tile([P, NKC, HPG, HD + 1], F16, name="v_sb")
        attnT = persist.tile([P, 2, S], F16, name="attnT")          # 8KB/part
        wout_sb = persist.tile([P, 2, D], F16, name="wout_sb")
        wqk_sb = persist.tile([P, NDC, 2 * CL], F16, name="wqk_sb")
        wv_sb = persist.tile([P, NDC, CL], F16, name="wv_sb")
        bqk_sb = persist.tile([P, 4], F32, name="bqk_sb")
        tri_sb = persist.tile([P, P], F16, name="tri_sb")
        ident = persist.tile([P, P], F16, name="ident")
        ones_bf = persist.tile([P, HD], BF16, name="ones_bf")

        # small constants FIRST on the fast Sync HWDGE ring: ident gates the
        # warm-up matmuls and the first transposes (the SWDGE ring takes
        # ~20us to deliver, stalling the whole pipeline start)
        nc.sync.dma_start(out=ident, in_=idn_d.ap())
        nc.sync.dma_start(out=tri_sb, in_=tri_d.ap())
        nc.sync.dma_start(out=bqk_sb, in_=bqk_d.ap())
        # weights on the ACT HWDGE ring (parallel with x loads on Sync ring)
        nc.scalar.dma_start(out=wqk_sb, in_=wqk_d.ap()
                            .rearrange("(c p) m -> p c m", p=P))
        nc.scalar.dma_start(out=wv_sb, in_=wv_d.ap()
                            .rearrange("(c p) m -> p c m", p=P))
        # stationary ones row (partition 64) for the 1/d broadcast matmul
        nc.gpsimd.memset(ones_bf[64:65, :], 1.0)
        # ones column of V tiles -> free softmax denominator row in PV
        for sc in range(NSC):
            nc.gpsimd.memset(v_sb[:, sc, :, HD], 1.0)
        # wout only needed from the first output projection (~20us in)
        nc.scalar.dma_start(out=wout_sb, in_=wout_d.ap()
                            .rearrange("(c p) o -> p c o", p=P))

        with tc.tile_pool(name="xTp", bufs=1) as xTp, \
                tc.tile_pool(name="xp", bufs=2) as xp:
            xT = xTp.tile([P, NDC, S], F16, name="xT")              # 32KB/part

            # x chunk DMAs: issue all up-front, alternating HWDGE rings
            xqs = []
            for scq in range(4):
                xq = xp.tile([P, 4, D], F16, tag="x", name=f"xq{scq}")
                eng = nc.sync if scq % 2 == 0 else nc.scalar
                eng.dma_start(
                    out=xq,
                    in_=x_d.ap()[scq * 512:(scq + 1) * 512, :]
                    .rearrange("(s p) d -> p s d", p=P))
                xqs.append(xq)

            # warm the ACT table (exp) during the initial DMA wait
            warm = tmpp.tile([HD, 512], F16, tag="tmp", name="warm")
            nc.scalar.activation(warm[0:1, 0:16], tri_sb[0:1, 0:16],
                                 mybir.ActivationFunctionType.Exp, scale=1.0)

            # warm the PE HAM clock-gate during the x DMA wait: ~45 dummy
            # back-to-back matmuls (~4.5us at the cold 1.2GHz clock) flip the
            # activity monitor to 8/8 before the real pipeline starts.
            wps = ps_sm.tile([P, 512], F32, tag="sm", name="wps")
            for w in range(45):
                nc.tensor.matmul(wps[:, 0:P], ident, ident,
                                 start=True, stop=True)

            def emit_outproj(qjo):
                """Output projection for q-span qjo (y stores alternate the
                Sync and SWDGE rings so the 8MB of stores never serialize on
                one ring at the tail)."""
                for si in range(4):
                    sc = 4 * qjo + si
                    y_sb = youtp.tile([P, D], F32, tag="y", name=f"y{sc}")
                    for oc in range(2):
                        py = ps_sm.tile([P, 512], F32, tag="sm",
                                        name=f"py{sc}_{oc}")
                        for cc in range(2):
                            nc.tensor.matmul(
                                py[:],
                                attnT[:, cc, sc * P:(sc + 1) * P],
                                wout_sb[:, cc, oc * 512:(oc + 1) * 512],
                                start=(cc == 0), stop=(cc == 1))
                        nc.vector.tensor_copy(
                            y_sb[:, oc * 512:(oc + 1) * 512], py[:])
                    eng = nc.sync if sc % 2 == 0 else nc.gpsimd
                    eng.dma_start(
                        out=y_d.ap()[sc * P:(sc + 1) * P, :], in_=y_sb)

            def emit_proj(qj):
                """Transpose x chunk qj into xT, then QK and V projection
                slices for that span (all full-util PE work)."""
                q0 = qj * 512
                xq = xqs[qj]
                for dc in range(NDC):
                    ptr = ps_sm.tile([P, 512], F16, tag="sm",
                                     name=f"ptr{qj}_{dc}")
                    for si in range(4):
                        nc.tensor.transpose(
                            ptr[:, si * P:(si + 1) * P],
                            xq[:, si, dc * P:(dc + 1) * P],
                            ident)
                    nc.vector.tensor_copy(
                        xT[:, dc, q0:q0 + 512], ptr)
                for mc in range(4):
                    pq = ps_sm.tile([P, 512], F32, tag="sm",
                                    name=f"pq{mc}_{qj}")
                    for kc in range(NDC):
                        nc.tensor.matmul(
                            pq[:],
                            wqk_sb[:, kc, mc * P:(mc + 1) * P],
                            xT[:, kc, q0:q0 + 512],
                            start=(kc == 0), stop=(kc == NDC - 1))
                    nc.vector.tensor_scalar_add(
                        qkt_sb[:, mc, q0:q0 + 512], pq[:],
                        bqk_sb[:, mc:mc + 1])
                for si in range(4):
                    sc = 4 * qj + si
                    pv = ps_sm.tile([P, CL], F32, tag="sm", name=f"pv{sc}")
                    for kc in range(NDC):
                        nc.tensor.matmul(
                            pv[:],
                            xT[:, kc, sc * P:(sc + 1) * P],
                            wv_sb[:, kc, :],
                            start=(kc == 0), stop=(kc == NDC - 1))
                    nc.vector.tensor_copy(
                        v_sb[:, sc, :, 0:HD],
                        pv.rearrange("p (h d) -> p h d", h=HPG))

            # Attention spans run in order 1,2,3,0: the long span-3 exp
            # chain gets output projections 1,2 as PE fill work, and the
            # tail is capped by the SHORT span-0 chain + outproj(0) rather
            # than span 3's. Prologue projects spans 0 and 1.
            emit_proj(0)
            emit_proj(1)

            for qj in (1, 2, 3, 0):
                q0 = qj * 512
                nkc = 4 * (qj + 1)

                # attention: head pairs (2p at rows 0..63, 2p+1 at 64..127)
                for p in range(2):
                    mcq, mck = p, 2 + p
                    hA, hB = 2 * p, 2 * p + 1
                    avA = ps_av.tile([P, 512], F32, tag="av",
                                     name=f"avA{qj}_{p}")
                    avB = ps_av.tile([P, 512], F32, tag="av",
                                     name=f"avB{qj}_{p}")
                    for kc in range(nkc):
                        t = kc - 4 * qj
                        c0 = 128 * t if t > 0 else 0
                        stp = ps_st.tile([P, 1024], F32, tag="st",
                                         name=f"st{qj}_{p}_{kc}")
                        # paired K=64 score matmuls on disjoint row groups:
                        # run concurrently on the PE array
                        nc.tensor.matmul(
                            stp[:, c0:512],
                            qkt_sb[0:HD, mck, kc * P:(kc + 1) * P],
                            qkt_sb[0:HD, mcq, q0 + c0:q0 + 512],
                            start=True, stop=True)
                        nc.tensor.matmul(
                            stp[:, 512 + c0:1024],
                            qkt_sb[HD:P, mck, kc * P:(kc + 1) * P],
                            qkt_sb[HD:P, mcq, q0 + c0:q0 + 512],
                            start=True, stop=True)
                        pt = ptp.tile([P, 1024], F16, tag="pt",
                                      name=f"pt{qj}_{p}_{kc}")
                        nc.scalar.activation(
                            pt[:, c0:1024], stp[:, c0:1024],
                            mybir.ActivationFunctionType.Exp, scale=0.125)
                        if 0 <= t <= 3:
                            nc.vector.tensor_mul(
                                pt[:, c0:c0 + 128],
                                pt[:, c0:c0 + 128], tri_sb)
                            nc.vector.tensor_mul(
                                pt[:, 512 + c0:512 + c0 + 128],
                                pt[:, 512 + c0:512 + c0 + 128], tri_sb)
                        nc.tensor.matmul(
                            avA[0:HD + 1, c0:512],
                            v_sb[:, kc, hA, :],
                            pt[:, c0:512],
                            start=(kc == 0), stop=(kc == nkc - 1))
                        nc.tensor.matmul(
                            avB[0:HD + 1, c0:512],
                            v_sb[:, kc, hB, :],
                            pt[:, 512 + c0:1024],
                            start=(kc == 0), stop=(kc == nkc - 1))

                    # normalize: evict both heads first (frees both av
                    # slots), then per head: bf16 denominator broadcast via
                    # K=1 matmul (dnb reuses the freed av slots, keeping
                    # ps_sm free of attention-phase allocations), reciprocal,
                    # multiply. Odd head first so its partition-shift DMA
                    # hides behind the even head's chain.
                    unB = unp.tile([HD + 1, 512], F32, tag="un",
                                   name=f"unB{qj}_{p}")
                    nc.vector.tensor_copy(unB, avB[0:HD + 1, :])
                    unA = unp.tile([HD + 1, 512], F32, tag="un",
                                   name=f"unA{qj}_{p}")
                    nc.vector.tensor_copy(unA, avA[0:HD + 1, :])
                    for un, h in ((unB, hB), (unA, hA)):
                        rcb = rcpb.tile([HD + 1, 512], BF16, tag="rcb",
                                        name=f"rcb{qj}_{h}")
                        nc.vector.tensor_copy(rcb[HD:HD + 1, :],
                                              un[HD:HD + 1, :])
                        dnb = ps_av.tile([P, 512], F32, tag="av",
                                         name=f"dnb{qj}_{h}")
                        nc.tensor.matmul(dnb[0:HD, :],
                                         ones_bf[HD:HD + 1, :],
                                         rcb[HD:HD + 1, :],
                                         start=True, stop=True)
                        rbs = rcpf.tile([HD, 512], F32, tag="rcf",
                                        name=f"rbs{qj}_{h}")
                        nc.vector.reciprocal_approx_fast(rbs, dnb[0:HD, :])
                        if h % 2 == 0:
                            nc.vector.tensor_mul(
                                attnT[0:HD, p, q0:q0 + 512],
                                un[0:HD, :], rbs)
                        else:
                            tmp = tmpp.tile([HD, 512], F16, tag="tmp",
                                            name=f"tmp{qj}_{h}")
                            nc.vector.tensor_mul(tmp, un[0:HD, :], rbs)
                            nc.sync.dma_start(
                                out=attnT[HD:P, p, q0:q0 + 512], in_=tmp)

                # fill work emitted AFTER this span's attention, so the
                # scheduler (priority = program order) runs it only when
                # attention matmuls are stalled on the exp stream.
                if qj == 1:
                    emit_proj(2)
                elif qj == 2:
                    emit_proj(3)
                    emit_outproj(1)
                elif qj == 3:
                    emit_outproj(2)
                else:  # qj == 0, last attention span
                    emit_outproj(3)
                    emit_outproj(0)

    nc.compile()
    _CACHED["nc"] = nc
    return nc


def _host_inputs(x, W_qkv, b_qkv):
    """Build the 8 per-core input maps (wout filled in by caller)."""
    x16 = np.asarray(x, dtype=np.float16)
    tri = (np.arange(P)[None, :] >= np.arange(P)[:, None]).astype(np.float16)
    in_maps = []
    for b in range(B):
        for hg in range(HG):
            c0 = hg * CL
            wqk = np.ascontiguousarray(
                np.concatenate([W_qkv[:, c0:c0 + CL],
                                W_qkv[:, D + c0:D + c0 + CL]], axis=1)
                .astype(np.float16))
            wv = np.ascontiguousarray(
                W_qkv[:, 2 * D + c0:2 * D + c0 + CL].astype(np.float16))
            bqk = np.ascontiguousarray(
                np.concatenate([b_qkv[c0:c0 + CL],
                                b_qkv[D + c0:D + c0 + CL]])
                .reshape(4, P).T, dtype=np.float32)
            in_maps.append({
                "x": x16[b], "wqk": wqk, "wv": wv, "wout": None,
                "bqk": bqk, "tri": tri, "idn": np.eye(P, dtype=np.float16),
            })
    return in_maps


def kernel(x, W_qkv, b_qkv, W_out, b_out):
    x = np.asarray(x, dtype=np.float32)
    W_qkv = np.asarray(W_qkv, dtype=np.float32)
    b_qkv = np.asarray(b_qkv, dtype=np.float32)
    W_out = np.asarray(W_out, dtype=np.float32)
    b_out = np.asarray(b_out, dtype=np.float32)

    nc = _build()
    in_maps = _host_inputs(x, W_qkv, b_qkv)
    for i, m in enumerate(in_maps):
        hg = i % HG
        m["wout"] = np.ascontiguousarray(
            W_out[hg * CL:(hg + 1) * CL, :].astype(np.float16))
    core_ids = list(range(8))
    res = run_bass_kernel_spmd(nc, in_maps, core_ids)
    outs = [r["y"] for r in res.results]
    bv = b_qkv[2 * D:3 * D]
    corr = (bv @ W_out + b_out).astype(np.float32)
    y = np.empty((B, S, D), dtype=np.float32)
    for b in range(B):
        acc = outs[b * HG].astype(np.float32).copy()
        for hg in range(1, HG):
            acc += outs[b * HG + hg]
        y[b] = acc + corr
    return y


# revision 23
# speedup vs baseline: 1.0342x; 1.0342x over previous
"""Causal self-attention Trainium2 kernel.

Reference (full): x[B=2,S=2048,D=1024] @ W_qkv + b_qkv -> 16-head causal
attention -> @ W_out + b_out.

Sharding: 8 cores = (batch b in 0..1) x (head-group hg in 0..3, 4 heads of
hd=64 each). Each core computes a partial output projection for its 4 heads
on its batch; the host sums the 4 head-group partials per batch and adds the
(constant) V-bias correction bv @ W_out and b_out.

Device pipeline per core (data path fp16, accumulation fp32 PSUM):
  The 4 local heads form 2 pairs (2p, 2p+1) whose q/k features live on
  partition halves 0..63 / 64..127 of the projected qkt tile.  Score
  matmuls for a pair are emitted back-to-back as K=64 row-tiled matmuls on
  disjoint row groups (tile_position (0,0) / (64,0)), so the PE array runs
  both heads CONCURRENTLY at full 128-row occupancy - 2x score throughput
  vs serial K=64 matmuls, and enough PE-cell activity to keep the HAM
  clock-gate at 8/8.
  Per (q-span, pair, k-chunk): paired score MMs -> one exp on ACT (scale
  1/8) -> triangle mask on diagonal blocks -> two PV matmuls (M=65: head
  dims + a ones column that yields the softmax denominator for free).
  Normalization: reciprocal of the denominator row on DVE, cast bf16, K=1
  bf16 matmul broadcasts 1/d across partitions (1 cyc/row vs 4 for f32),
  multiply on eviction; odd heads staged through SBUF and DMA'd to
  partitions 64..127.
  x-transposes for q-span j are emitted just before span j's work so the
  first attention block starts ~3us in; projections / output projections
  interleave into the ACT-bound attention stream via the tile scheduler.
"""
import numpy as np
from contextlib import ExitStack

import concourse.bacc as bacc
import concourse.tile as tile
from concourse import mybir
from concourse.bass_utils import run_bass_kernel_spmd

F32 = mybir.dt.float32
BF16 = mybir.dt.bfloat16
F16 = mybir.dt.float16

B = 2
S = 2048
D = 1024
HD = 64
HG = 4            # head-groups (cores per batch)
HPG = 4           # heads per group
CL = HPG * HD     # 256 local head cols per core
P = 128
NSC = S // P      # 16 s-chunks
NDC = D // P      # 8 d-chunks
NQJ = S // 512    # 4 q-spans
NKC = S // P      # 16 k-chunks

_CACHED = {}


def _build():
    if "nc" in _CACHED:
        return _CACHED["nc"]
    nc = bacc.Bacc("TRN2", target_bir_lowering=False, debug=False)

    x_d = nc.dram_tensor("x", [S, D], F16, kind="ExternalInput")
    wqk_d = nc.dram_tensor("wqk", [D, 2 * CL], F16, kind="ExternalInput")
    wv_d = nc.dram_tensor("wv", [D, CL], F16, kind="ExternalInput")
    wout_d = nc.dram_tensor("wout", [CL, D], F16, kind="ExternalInput")
    bqk_d = nc.dram_tensor("bqk", [P, 4], F32, kind="ExternalInput")
    tri_d = nc.dram_tensor("tri", [P, P], F16, kind="ExternalInput")
    idn_d = nc.dram_tensor("idn", [P, P], F16, kind="ExternalInput")
    y_d = nc.dram_tensor("y", [S, D], F32, kind="ExternalOutput")

    with tile.TileContext(nc) as tc, ExitStack() as ctx:
        persist = ctx.enter_context(tc.tile_pool(name="persist", bufs=1))
        ptp = ctx.enter_context(tc.tile_pool(name="ptp", bufs=3))
        youtp = ctx.enter_context(tc.tile_pool(name="youtp", bufs=2))
        unp = ctx.enter_context(tc.tile_pool(name="unp", bufs=2))
        rcpf = ctx.enter_context(tc.tile_pool(name="rcpf", bufs=2))
        rcpb = ctx.enter_context(tc.tile_pool(name="rcpb", bufs=2))
        tmpp = ctx.enter_context(tc.tile_pool(name="tmpp", bufs=2))
        ps_sm = ctx.enter_context(tc.tile_pool(name="ps_sm", bufs=2, space="PSUM"))
        ps_st = ctx.enter_context(tc.tile_pool(name="ps_st", bufs=2, space="PSUM"))
        ps_av = ctx.enter_context(tc.tile_pool(name="ps_av", bufs=2, space="PSUM"))

        # ---- persistent tiles ----
        qkt_sb = persist.tile([P, 4, S], F16, name="qkt_sb")        # 16KB/part
        v_sb = persist.tile([P, NKC, HPG, HD + 1], F16, name="v_sb")
        attnT = persist.tile([P, 2, S], F16, name="attnT")          # 8KB/part
        wout_sb = persist.tile([P, 2, D], F16, name="wout_sb")
        wqk_sb = persist.tile([P, NDC, 2 * CL], F16, name="wqk_sb")
        wv_sb = persist.tile([P, NDC, CL], F16, name="wv_sb")
        bqk_sb = persist.tile([P, 4], F32, name="bqk_sb")
        tri_sb = persist.tile([P, P], F16, name="tri_sb")
        ident = persist.tile([P, P], F16, name="ident")
        ones_bf = persist.tile([P, HD], BF16, name="ones_bf")
        wsrc = persist.tile([P, P], F16, name="wsrc")

        # small constants FIRST on the fast Sync HWDGE ring: ident gates the
        # warm-up matmuls and the first transposes (the SWDGE ring takes
        # ~20us to deliver, stalling the whole pipeline start)
        nc.sync.dma_start(out=ident, in_=idn_d.ap())
        nc.sync.dma_start(out=tri_sb, in_=tri_d.ap())
        nc.sync.dma_start(out=bqk_sb, in_=bqk_d.ap())
        # weights on the ACT HWDGE ring (parallel with x loads on Sync ring)
        nc.scalar.dma_start(out=wqk_sb, in_=wqk_d.ap()
                            .rearrange("(c p) m -> p c m", p=P))
        nc.scalar.dma_start(out=wv_sb, in_=wv_d.ap()
                            .rearrange("(c p) m -> p c m", p=P))
        # stationary ones row (partition 64) for the 1/d broadcast matmul
        nc.gpsimd.memset(ones_bf[64:65, :], 1.0)
        # ones column of V tiles -> free softmax denominator row in PV
        for sc in range(NSC):
            nc.gpsimd.memset(v_sb[:, sc, :, HD], 1.0)
        # wout only needed from the first output projection (~20us in)
        nc.scalar.dma_start(out=wout_sb, in_=wout_d.ap()
                            .rearrange("(c p) o -> p c o", p=P))

        with tc.tile_pool(name="xTp", bufs=1) as xTp, \
                tc.tile_pool(name="xp", bufs=2) as xp:
            xT = xTp.tile([P, NDC, S], F16, name="xT")              # 32KB/part

            # x chunk DMAs: issue all up-front, alternating HWDGE rings
            xqs = []
            for scq in range(4):
                xq = xp.tile([P, 4, D], F16, tag="x", name=f"xq{scq}")
                eng = nc.sync if scq % 2 == 0 else nc.scalar
                eng.dma_start(
                    out=xq,
                    in_=x_d.ap()[scq * 512:(scq + 1) * 512, :]
                    .rearrange("(s p) d -> p s d", p=P))
                xqs.append(xq)

            # warm the ACT table (exp) during the initial DMA wait
            warm = tmpp.tile([HD, 512], F16, tag="tmp", name="warm")
            nc.scalar.activation(warm[0:1, 0:16], tri_sb[0:1, 0:16],
                                 mybir.ActivationFunctionType.Exp, scale=1.0)

            # warm the PE HAM clock-gate during the x DMA wait: dummy
            # back-to-back matmuls on a memset tile (no DMA dependency -
            # the HWDGE rings deliver first bytes only ~10us in) flip the
            # activity monitor to 8/8 before the real pipeline starts.
            nc.vector.memset(wsrc, 0.5)
            wps = ps_sm.tile([P, 512], F32, tag="sm", name="wps")
            for w in range(45):
                nc.tensor.matmul(wps[:, 0:P], wsrc, wsrc,
                                 start=True, stop=True)

            def emit_outproj(qjo):
                """Output projection for q-span qjo (y stores alternate the
                Sync and SWDGE rings so the 8MB of stores never serialize on
                one ring at the tail)."""
                for si in range(4):
                    sc = 4 * qjo + si
                    y_sb = youtp.tile([P, D], F32, tag="y", name=f"y{sc}")
                    for oc in range(2):
                        py = ps_sm.tile([P, 512], F32, tag="sm",
                                        name=f"py{sc}_{oc}")
                        for cc in range(2):
                            nc.tensor.matmul(
                                py[:],
                                attnT[:, cc, sc * P:(sc + 1) * P],
                                wout_sb[:, cc, oc * 512:(oc + 1) * 512],
                                start=(cc == 0), stop=(cc == 1))
                        nc.vector.tensor_copy(
                            y_sb[:, oc * 512:(oc + 1) * 512], py[:])
                    eng = nc.sync if sc % 2 == 0 else nc.gpsimd
                    eng.dma_start(
                        out=y_d.ap()[sc * P:(sc + 1) * P, :], in_=y_sb)

            def emit_proj(qj):
                """Transpose x chunk qj into xT, then QK and V projection
                slices for that span (all full-util PE work)."""
                q0 = qj * 512
                xq = xqs[qj]
                for dc in range(NDC):
                    ptr = ps_sm.tile([P, 512], F16, tag="sm",
                                     name=f"ptr{qj}_{dc}")
                    for si in range(4):
                        nc.tensor.transpose(
                            ptr[:, si * P:(si + 1) * P],
                            xq[:, si, dc * P:(dc + 1) * P],
                            ident)
                    nc.vector.tensor_copy(
                        xT[:, dc, q0:q0 + 512], ptr)
                for mc in range(4):
                    pq = ps_sm.tile([P, 512], F32, tag="sm",
                                    name=f"pq{mc}_{qj}")
                    for kc in range(NDC):
                        nc.tensor.matmul(
                            pq[:],
                            wqk_sb[:, kc, mc * P:(mc + 1) * P],
                            xT[:, kc, q0:q0 + 512],
                            start=(kc == 0), stop=(kc == NDC - 1))
                    nc.vector.tensor_scalar_add(
                        qkt_sb[:, mc, q0:q0 + 512], pq[:],
                        bqk_sb[:, mc:mc + 1])
                for si in range(4):
                    sc = 4 * qj + si
                    pv = ps_sm.tile([P, CL], F32, tag="sm", name=f"pv{sc}")
                    for kc in range(NDC):
                        nc.tensor.matmul(
                            pv[:],
                            xT[:, kc, sc * P:(sc + 1) * P],
                            wv_sb[:, kc, :],
                            start=(kc == 0), stop=(kc == NDC - 1))
                    nc.vector.tensor_copy(
                        v_sb[:, sc, :, 0:HD],
                        pv.rearrange("p (h d) -> p h d", h=HPG))

            # prologue: span-0 projections (attention(0) needs them)
            emit_proj(0)

            for qj in range(NQJ):
                q0 = qj * 512
                nkc = 4 * (qj + 1)

                # attention: head pairs (2p at rows 0..63, 2p+1 at 64..127)
                for p in range(2):
                    mcq, mck = p, 2 + p
                    hA, hB = 2 * p, 2 * p + 1
                    avA = ps_av.tile([P, 512], F32, tag="av",
                                     name=f"avA{qj}_{p}")
                    avB = ps_av.tile([P, 512], F32, tag="av",
                                     name=f"avB{qj}_{p}")
                    for kc in range(nkc):
                        t = kc - 4 * qj
                        c0 = 128 * t if t > 0 else 0
                        stp = ps_st.tile([P, 1024], F32, tag="st",
                                         name=f"st{qj}_{p}_{kc}")
                        # paired K=64 score matmuls on disjoint row groups:
                        # run concurrently on the PE array
                        nc.tensor.matmul(
                            stp[:, c0:512],
                            qkt_sb[0:HD, mck, kc * P:(kc + 1) * P],
                            qkt_sb[0:HD, mcq, q0 + c0:q0 + 512],
                            start=True, stop=True)
                        nc.tensor.matmul(
                            stp[:, 512 + c0:1024],
                            qkt_sb[HD:P, mck, kc * P:(kc + 1) * P],
                            qkt_sb[HD:P, mcq, q0 + c0:q0 + 512],
                            start=True, stop=True)
                        pt = ptp.tile([P, 1024], F16, tag="pt",
                                      name=f"pt{qj}_{p}_{kc}")
                        nc.scalar.activation(
                            pt[:, c0:1024], stp[:, c0:1024],
                            mybir.ActivationFunctionType.Exp, scale=0.125)
                        if 0 <= t <= 3:
                            nc.vector.tensor_mul(
                                pt[:, c0:c0 + 128],
                                pt[:, c0:c0 + 128], tri_sb)
                            nc.vector.tensor_mul(
                                pt[:, 512 + c0:512 + c0 + 128],
                                pt[:, 512 + c0:512 + c0 + 128], tri_sb)
                        nc.tensor.matmul(
                            avA[0:HD + 1, c0:512],
                            v_sb[:, kc, hA, :],
                            pt[:, c0:512],
                            start=(kc == 0), stop=(kc == nkc - 1))
                        nc.tensor.matmul(
                            avB[0:HD + 1, c0:512],
                            v_sb[:, kc, hB, :],
                            pt[:, 512 + c0:1024],
                            start=(kc == 0), stop=(kc == nkc - 1))

                    # normalize: evict both heads first (frees both av
                    # slots), then per head: bf16 denominator broadcast via
                    # K=1 matmul (dnb reuses the freed av slots, keeping
                    # ps_sm free of attention-phase allocations), reciprocal,
                    # multiply. Odd head first so its partition-shift DMA
                    # hides behind the even head's chain.
                    unB = unp.tile([HD + 1, 512], F32, tag="un",
                                   name=f"unB{qj}_{p}")
                    nc.vector.tensor_copy(unB, avB[0:HD + 1, :])
                    unA = unp.tile([HD + 1, 512], F32, tag="un",
                                   name=f"unA{qj}_{p}")
                    nc.vector.tensor_copy(unA, avA[0:HD + 1, :])
                    for un, h in ((unB, hB), (unA, hA)):
                        rcb = rcpb.tile([HD + 1, 512], BF16, tag="rcb",
                                        name=f"rcb{qj}_{h}")
                        nc.vector.tensor_copy(rcb[HD:HD + 1, :],
                                              un[HD:HD + 1, :])
                        dnb = ps_av.tile([P, 512], F32, tag="av",
                                         name=f"dnb{qj}_{h}")
                        nc.tensor.matmul(dnb[0:HD, :],
                                         ones_bf[HD:HD + 1, :],
                                         rcb[HD:HD + 1, :],
                                         start=True, stop=True)
                        rbs = rcpf.tile([HD, 512], F32, tag="rcf",
                                        name=f"rbs{qj}_{h}")
                        nc.vector.reciprocal_approx_fast(rbs, dnb[0:HD, :])
                        if h % 2 == 0:
                            nc.vector.tensor_mul(
                                attnT[0:HD, p, q0:q0 + 512],
                                un[0:HD, :], rbs)
                        else:
                            tmp = tmpp.tile([HD, 512], F16, tag="tmp",
                                            name=f"tmp{qj}_{h}")
                            nc.vector.tensor_mul(tmp, un[0:HD, :], rbs)
                            nc.sync.dma_start(
                                out=attnT[HD:P, p, q0:q0 + 512], in_=tmp)

                # fill work emitted AFTER this span's attention, so the
                # scheduler (priority = program order) runs it only when
                # attention matmuls are stalled on the exp stream: next
                # span's projections; in the last span - where no proj work
                # remains - all four output projections.
                if qj < NQJ - 1:
                    emit_proj(qj + 1)
                else:
                    for qjo in range(NQJ):
                        emit_outproj(qjo)

    nc.compile()
    _CACHED["nc"] = nc
    return nc


def _host_inputs(x, W_qkv, b_qkv):
    """Build the 8 per-core input maps (wout filled in by caller)."""
    x16 = np.asarray(x, dtype=np.float16)
    tri = (np.arange(P)[None, :] >= np.arange(P)[:, None]).astype(np.float16)
    in_maps = []
    for b in range(B):
        for hg in range(HG):
            c0 = hg * CL
            wqk = np.ascontiguousarray(
                np.concatenate([W_qkv[:, c0:c0 + CL],
                                W_qkv[:, D + c0:D + c0 + CL]], axis=1)
                .astype(np.float16))
            wv = np.ascontiguousarray(
                W_qkv[:, 2 * D + c0:2 * D + c0 + CL].astype(np.float16))
            bqk = np.ascontiguousarray(
                np.concatenate([b_qkv[c0:c0 + CL],
                                b_qkv[D + c0:D + c0 + CL]])
                .reshape(4, P).T, dtype=np.float32)
            in_maps.append({
                "x": x16[b], "wqk": wqk, "wv": wv, "wout": None,
                "bqk": bqk, "tri": tri, "idn": np.eye(P, dtype=np.float16),
            })
    return in_maps


def kernel(x, W_qkv, b_qkv, W_out, b_out):
    x = np.asarray(x, dtype=np.float32)
    W_qkv = np.asarray(W_qkv, dtype=np.float32)
    b_qkv = np.asarray(b_qkv, dtype=np.float32)
    W_out = np.asarray(W_out, dtype=np.float32)
    b_out = np.asarray(b_out, dtype=np.float32)

    nc = _build()
    in_maps = _host_inputs(x, W_qkv, b_qkv)
    for i, m in enumerate(in_maps):
        hg = i % HG
        m["wout"] = np.ascontiguousarray(
            W_out[hg * CL:(hg + 1) * CL, :].astype(np.float16))
    core_ids = list(range(8))
    res = run_bass_kernel_spmd(nc, in_maps, core_ids)
    outs = [r["y"] for r in res.results]
    bv = b_qkv[2 * D:3 * D]
    corr = (bv @ W_out + b_out).astype(np.float32)
    y = np.empty((B, S, D), dtype=np.float32)
    for b in range(B):
        acc = outs[b * HG].astype(np.float32).copy()
        for hg in range(1, HG):
            acc += outs[b * HG + hg]
        y[b] = acc + corr
    return y


# revision 24
# speedup vs baseline: 1.0375x; 1.0031x over previous
"""Causal self-attention Trainium2 kernel.

Reference (full): x[B=2,S=2048,D=1024] @ W_qkv + b_qkv -> 16-head causal
attention -> @ W_out + b_out.

Sharding: 8 cores = (batch b in 0..1) x (head-group hg in 0..3, 4 heads of
hd=64 each). Each core computes a partial output projection for its 4 heads
on its batch; the host sums the 4 head-group partials per batch and adds the
(constant) V-bias correction bv @ W_out and b_out.

Device pipeline per core (data path fp16, accumulation fp32 PSUM):
  The 4 local heads form 2 pairs (2p, 2p+1) whose q/k features live on
  partition halves 0..63 / 64..127 of the projected qkt tile.  Score
  matmuls for a pair are emitted back-to-back as K=64 row-tiled matmuls on
  disjoint row groups (tile_position (0,0) / (64,0)), so the PE array runs
  both heads CONCURRENTLY at full 128-row occupancy - 2x score throughput
  vs serial K=64 matmuls, and enough PE-cell activity to keep the HAM
  clock-gate at 8/8.
  Per (q-span, pair, k-chunk): paired score MMs -> one exp on ACT (scale
  1/8) -> triangle mask on diagonal blocks -> two PV matmuls (M=65: head
  dims + a ones column that yields the softmax denominator for free).
  Normalization: reciprocal of the denominator row on DVE, cast bf16, K=1
  bf16 matmul broadcasts 1/d across partitions (1 cyc/row vs 4 for f32),
  multiply on eviction; odd heads staged through SBUF and DMA'd to
  partitions 64..127.
  x-transposes for q-span j are emitted just before span j's work so the
  first attention block starts ~3us in; projections / output projections
  interleave into the ACT-bound attention stream via the tile scheduler.
"""
import numpy as np
from contextlib import ExitStack

import concourse.bacc as bacc
import concourse.tile as tile
from concourse import mybir
from concourse.bass_utils import run_bass_kernel_spmd

F32 = mybir.dt.float32
BF16 = mybir.dt.bfloat16
F16 = mybir.dt.float16

B = 2
S = 2048
D = 1024
HD = 64
HG = 4            # head-groups (cores per batch)
HPG = 4           # heads per group
CL = HPG * HD     # 256 local head cols per core
P = 128
NSC = S // P      # 16 s-chunks
NDC = D // P      # 8 d-chunks
NQJ = S // 512    # 4 q-spans
NKC = S // P      # 16 k-chunks

_CACHED = {}


def _build():
    if "nc" in _CACHED:
        return _CACHED["nc"]
    nc = bacc.Bacc("TRN2", target_bir_lowering=False, debug=False)

    x_d = nc.dram_tensor("x", [S, D], F16, kind="ExternalInput")
    wqk_d = nc.dram_tensor("wqk", [D, 2 * CL], F16, kind="ExternalInput")
    wv_d = nc.dram_tensor("wv", [D, CL], F16, kind="ExternalInput")
    wout_d = nc.dram_tensor("wout", [CL, D], F16, kind="ExternalInput")
    bqk_d = nc.dram_tensor("bqk", [P, 4], F32, kind="ExternalInput")
    tri_d = nc.dram_tensor("tri", [P, P], F16, kind="ExternalInput")
    idn_d = nc.dram_tensor("idn", [P, P], F16, kind="ExternalInput")
    y_d = nc.dram_tensor("y", [S, D], F32, kind="ExternalOutput")

    with tile.TileContext(nc) as tc, ExitStack() as ctx:
        persist = ctx.enter_context(tc.tile_pool(name="persist", bufs=1))
        ptp = ctx.enter_context(tc.tile_pool(name="ptp", bufs=3))
        youtp = ctx.enter_context(tc.tile_pool(name="youtp", bufs=2))
        unp = ctx.enter_context(tc.tile_pool(name="unp", bufs=2))
        rcpf = ctx.enter_context(tc.tile_pool(name="rcpf", bufs=2))
        rcpb = ctx.enter_context(tc.tile_pool(name="rcpb", bufs=2))
        tmpp = ctx.enter_context(tc.tile_pool(name="tmpp", bufs=2))
        ps_sm = ctx.enter_context(tc.tile_pool(name="ps_sm", bufs=2, space="PSUM"))
        ps_st = ctx.enter_context(tc.tile_pool(name="ps_st", bufs=2, space="PSUM"))
        ps_av = ctx.enter_context(tc.tile_pool(name="ps_av", bufs=2, space="PSUM"))

        # ---- persistent tiles ----
        qkt_sb = persist.tile([P, 4, S], F16, name="qkt_sb")        # 16KB/part
        v_sb = persist.tile([P, NKC, HPG, HD + 1], F16, name="v_sb")
        attnT = persist.tile([P, 2, S], F16, name="attnT")          # 8KB/part
        wout_sb = persist.tile([P, 2, D], F16, name="wout_sb")
        wqk_sb = persist.tile([P, NDC, 2 * CL], F16, name="wqk_sb")
        wv_sb = persist.tile([P, NDC, CL], F16, name="wv_sb")
        bqk_sb = persist.tile([P, 4], F32, name="bqk_sb")
        tri_sb = persist.tile([P, P], F16, name="tri_sb")
        ident = persist.tile([P, P], F16, name="ident")
        ones_bf = persist.tile([P, HD], BF16, name="ones_bf")
        wsrc = persist.tile([P, P], F16, name="wsrc")

        # small constants FIRST on the fast Sync HWDGE ring: ident gates the
        # warm-up matmuls and the first transposes (the SWDGE ring takes
        # ~20us to deliver, stalling the whole pipeline start)
        nc.sync.dma_start(out=ident, in_=idn_d.ap())
        nc.sync.dma_start(out=tri_sb, in_=tri_d.ap())
        nc.sync.dma_start(out=bqk_sb, in_=bqk_d.ap())
        # weights on the ACT HWDGE ring (parallel with x loads on Sync ring)
        nc.scalar.dma_start(out=wqk_sb, in_=wqk_d.ap()
                            .rearrange("(c p) m -> p c m", p=P))
        nc.scalar.dma_start(out=wv_sb, in_=wv_d.ap()
                            .rearrange("(c p) m -> p c m", p=P))
        # stationary ones row (partition 64) for the 1/d broadcast matmul
        nc.gpsimd.memset(ones_bf[64:65, :], 1.0)
        # ones column of V tiles -> free softmax denominator row in PV
        for sc in range(NSC):
            nc.gpsimd.memset(v_sb[:, sc, :, HD], 1.0)
        # wout only needed from the first output projection (~20us in)
        nc.scalar.dma_start(out=wout_sb, in_=wout_d.ap()
                            .rearrange("(c p) o -> p c o", p=P))

        with tc.tile_pool(name="xTp", bufs=1) as xTp, \
                tc.tile_pool(name="xp", bufs=2) as xp:
            xT = xTp.tile([P, NDC, S], F16, name="xT")              # 32KB/part

            # x chunk DMAs: issue all up-front, alternating HWDGE rings
            xqs = []
            for scq in range(4):
                xq = xp.tile([P, 4, D], F16, tag="x", name=f"xq{scq}")
                eng = nc.sync if scq % 2 == 0 else nc.scalar
                eng.dma_start(
                    out=xq,
                    in_=x_d.ap()[scq * 512:(scq + 1) * 512, :]
                    .rearrange("(s p) d -> p s d", p=P))
                xqs.append(xq)

            # warm the ACT table (exp) during the initial DMA wait
            warm = tmpp.tile([HD, 512], F16, tag="tmp", name="warm")
            nc.scalar.activation(warm[0:1, 0:16], tri_sb[0:1, 0:16],
                                 mybir.ActivationFunctionType.Exp, scale=1.0)

            # warm the PE HAM clock-gate during the x DMA wait: dummy
            # back-to-back matmuls on a memset tile (no DMA dependency -
            # the HWDGE rings deliver first bytes only ~10us in) flip the
            # activity monitor to 8/8 before the real pipeline starts.
            # wps lives in ps_st (first real use ~20us in) so the warm-up
            # WAW chain never gates the ps_sm rotation of the prologue.
            nc.vector.memset(wsrc, 0.5)
            wps = ps_st.tile([P, 1024], F32, tag="st", name="wps")
            for w in range(45):
                nc.tensor.matmul(wps[:, 0:P], wsrc, wsrc,
                                 start=True, stop=True)

            def emit_outproj(qjo):
                """Output projection for q-span qjo (y stores alternate the
                Sync and SWDGE rings so the 8MB of stores never serialize on
                one ring at the tail)."""
                for si in range(4):
                    sc = 4 * qjo + si
                    y_sb = youtp.tile([P, D], F32, tag="y", name=f"y{sc}")
                    for oc in range(2):
                        py = ps_sm.tile([P, 512], F32, tag="sm",
                                        name=f"py{sc}_{oc}")
                        for cc in range(2):
                            nc.tensor.matmul(
                                py[:],
                                attnT[:, cc, sc * P:(sc + 1) * P],
                                wout_sb[:, cc, oc * 512:(oc + 1) * 512],
                                start=(cc == 0), stop=(cc == 1))
                        nc.vector.tensor_copy(
                            y_sb[:, oc * 512:(oc + 1) * 512], py[:])
                    eng = nc.sync if sc % 2 == 0 else nc.gpsimd
                    eng.dma_start(
                        out=y_d.ap()[sc * P:(sc + 1) * P, :], in_=y_sb)

            def emit_proj(qj):
                """Transpose x chunk qj into xT, then QK and V projection
                slices for that span (all full-util PE work)."""
                q0 = qj * 512
                xq = xqs[qj]
                for dc in range(NDC):
                    ptr = ps_sm.tile([P, 512], F16, tag="sm",
                                     name=f"ptr{qj}_{dc}")
                    for si in range(4):
                        nc.tensor.transpose(
                            ptr[:, si * P:(si + 1) * P],
                            xq[:, si, dc * P:(dc + 1) * P],
                            ident)
                    nc.vector.tensor_copy(
                        xT[:, dc, q0:q0 + 512], ptr)
                for mc in range(4):
                    pq = ps_sm.tile([P, 512], F32, tag="sm",
                                    name=f"pq{mc}_{qj}")
                    for kc in range(NDC):
                        nc.tensor.matmul(
                            pq[:],
                            wqk_sb[:, kc, mc * P:(mc + 1) * P],
                            xT[:, kc, q0:q0 + 512],
                            start=(kc == 0), stop=(kc == NDC - 1))
                    nc.vector.tensor_scalar_add(
                        qkt_sb[:, mc, q0:q0 + 512], pq[:],
                        bqk_sb[:, mc:mc + 1])
                for si in range(4):
                    sc = 4 * qj + si
                    pv = ps_sm.tile([P, CL], F32, tag="sm", name=f"pv{sc}")
                    for kc in range(NDC):
                        nc.tensor.matmul(
                            pv[:],
                            xT[:, kc, sc * P:(sc + 1) * P],
                            wv_sb[:, kc, :],
                            start=(kc == 0), stop=(kc == NDC - 1))
                    nc.vector.tensor_copy(
                        v_sb[:, sc, :, 0:HD],
                        pv.rearrange("p (h d) -> p h d", h=HPG))

            # prologue: span-0 projections (attention(0) needs them)
            emit_proj(0)

            for qj in range(NQJ):
                q0 = qj * 512
                nkc = 4 * (qj + 1)

                # attention: head pairs (2p at rows 0..63, 2p+1 at 64..127)
                for p in range(2):
                    mcq, mck = p, 2 + p
                    hA, hB = 2 * p, 2 * p + 1
                    avA = ps_av.tile([P, 512], F32, tag="av",
                                     name=f"avA{qj}_{p}")
                    avB = ps_av.tile([P, 512], F32, tag="av",
                                     name=f"avB{qj}_{p}")
                    for kc in range(nkc):
                        t = kc - 4 * qj
                        c0 = 128 * t if t > 0 else 0
                        stp = ps_st.tile([P, 1024], F32, tag="st",
                                         name=f"st{qj}_{p}_{kc}")
                        # paired K=64 score matmuls on disjoint row groups:
                        # run concurrently on the PE array
                        nc.tensor.matmul(
                            stp[:, c0:512],
                            qkt_sb[0:HD, mck, kc * P:(kc + 1) * P],
                            qkt_sb[0:HD, mcq, q0 + c0:q0 + 512],
                            start=True, stop=True)
                        nc.tensor.matmul(
                            stp[:, 512 + c0:1024],
                            qkt_sb[HD:P, mck, kc * P:(kc + 1) * P],
                            qkt_sb[HD:P, mcq, q0 + c0:q0 + 512],
                            start=True, stop=True)
                        pt = ptp.tile([P, 1024], F16, tag="pt",
                                      name=f"pt{qj}_{p}_{kc}")
                        nc.scalar.activation(
                            pt[:, c0:1024], stp[:, c0:1024],
                            mybir.ActivationFunctionType.Exp, scale=0.125)
                        if 0 <= t <= 3:
                            nc.vector.tensor_mul(
                                pt[:, c0:c0 + 128],
                                pt[:, c0:c0 + 128], tri_sb)
                            nc.vector.tensor_mul(
                                pt[:, 512 + c0:512 + c0 + 128],
                                pt[:, 512 + c0:512 + c0 + 128], tri_sb)
                        nc.tensor.matmul(
                            avA[0:HD + 1, c0:512],
                            v_sb[:, kc, hA, :],
                            pt[:, c0:512],
                            start=(kc == 0), stop=(kc == nkc - 1))
                        nc.tensor.matmul(
                            avB[0:HD + 1, c0:512],
                            v_sb[:, kc, hB, :],
                            pt[:, 512 + c0:1024],
                            start=(kc == 0), stop=(kc == nkc - 1))

                    # normalize: evict both heads first (frees both av
                    # slots), then per head: bf16 denominator broadcast via
                    # K=1 matmul (dnb reuses the freed av slots, keeping
                    # ps_sm free of attention-phase allocations), reciprocal,
                    # multiply. Odd head first so its partition-shift DMA
                    # hides behind the even head's chain.
                    unB = unp.tile([HD + 1, 512], F32, tag="un",
                                   name=f"unB{qj}_{p}")
                    nc.vector.tensor_copy(unB, avB[0:HD + 1, :])
                    unA = unp.tile([HD + 1, 512], F32, tag="un",
                                   name=f"unA{qj}_{p}")
                    nc.vector.tensor_copy(unA, avA[0:HD + 1, :])
                    for un, h in ((unB, hB), (unA, hA)):
                        rcb = rcpb.tile([HD + 1, 512], BF16, tag="rcb",
                                        name=f"rcb{qj}_{h}")
                        nc.vector.tensor_copy(rcb[HD:HD + 1, :],
                                              un[HD:HD + 1, :])
                        dnb = ps_av.tile([P, 512], F32, tag="av",
                                         name=f"dnb{qj}_{h}")
                        nc.tensor.matmul(dnb[0:HD, :],
                                         ones_bf[HD:HD + 1, :],
                                         rcb[HD:HD + 1, :],
                                         start=True, stop=True)
                        rbs = rcpf.tile([HD, 512], F32, tag="rcf",
                                        name=f"rbs{qj}_{h}")
                        nc.vector.reciprocal_approx_fast(rbs, dnb[0:HD, :])
                        if h % 2 == 0:
                            nc.vector.tensor_mul(
                                attnT[0:HD, p, q0:q0 + 512],
                                un[0:HD, :], rbs)
                        else:
                            tmp = tmpp.tile([HD, 512], F16, tag="tmp",
                                            name=f"tmp{qj}_{h}")
                            nc.vector.tensor_mul(tmp, un[0:HD, :], rbs)
                            nc.sync.dma_start(
                                out=attnT[HD:P, p, q0:q0 + 512], in_=tmp)

                # fill work emitted AFTER this span's attention, so the
                # scheduler (priority = program order) runs it only when
                # attention matmuls are stalled on the exp stream: next
                # span's projections; in the last span - where no proj work
                # remains - all four output projections.
                if qj < NQJ - 1:
                    emit_proj(qj + 1)
                else:
                    for qjo in range(NQJ):
                        emit_outproj(qjo)

    nc.compile()
    _CACHED["nc"] = nc
    return nc


def _host_inputs(x, W_qkv, b_qkv):
    """Build the 8 per-core input maps (wout filled in by caller)."""
    x16 = np.asarray(x, dtype=np.float16)
    tri = (np.arange(P)[None, :] >= np.arange(P)[:, None]).astype(np.float16)
    in_maps = []
    for b in range(B):
        for hg in range(HG):
            c0 = hg * CL
            wqk = np.ascontiguousarray(
                np.concatenate([W_qkv[:, c0:c0 + CL],
                                W_qkv[:, D + c0:D + c0 + CL]], axis=1)
                .astype(np.float16))
            wv = np.ascontiguousarray(
                W_qkv[:, 2 * D + c0:2 * D + c0 + CL].astype(np.float16))
            bqk = np.ascontiguousarray(
                np.concatenate([b_qkv[c0:c0 + CL],
                                b_qkv[D + c0:D + c0 + CL]])
                .reshape(4, P).T, dtype=np.float32)
            in_maps.append({
                "x": x16[b], "wqk": wqk, "wv": wv, "wout": None,
                "bqk": bqk, "tri": tri, "idn": np.eye(P, dtype=np.float16),
            })
    return in_maps


def kernel(x, W_qkv, b_qkv, W_out, b_out):
    x = np.asarray(x, dtype=np.float32)
    W_qkv = np.asarray(W_qkv, dtype=np.float32)
    b_qkv = np.asarray(b_qkv, dtype=np.float32)
    W_out = np.asarray(W_out, dtype=np.float32)
    b_out = np.asarray(b_out, dtype=np.float32)

    nc = _build()
    in_maps = _host_inputs(x, W_qkv, b_qkv)
    for i, m in enumerate(in_maps):
        hg = i % HG
        m["wout"] = np.ascontiguousarray(
            W_out[hg * CL:(hg + 1) * CL, :].astype(np.float16))
    core_ids = list(range(8))
    res = run_bass_kernel_spmd(nc, in_maps, core_ids)
    outs = [r["y"] for r in res.results]
    bv = b_qkv[2 * D:3 * D]
    corr = (bv @ W_out + b_out).astype(np.float32)
    y = np.empty((B, S, D), dtype=np.float32)
    for b in range(B):
        acc = outs[b * HG].astype(np.float32).copy()
        for hg in range(1, HG):
            acc += outs[b * HG + hg]
        y[b] = acc + corr
    return y


# revision 27
# speedup vs baseline: 1.0510x; 1.0130x over previous
"""Causal self-attention Trainium2 kernel.

Reference (full): x[B=2,S=2048,D=1024] @ W_qkv + b_qkv -> 16-head causal
attention -> @ W_out + b_out.

Sharding: 8 cores = (batch b in 0..1) x (head-group hg in 0..3, 4 heads of
hd=64 each). Each core computes a partial output projection for its 4 heads
on its batch; the host sums the 4 head-group partials per batch and adds the
(constant) V-bias correction bv @ W_out and b_out.

Device pipeline per core (data path fp16, accumulation fp32 PSUM):
  The 4 local heads form 2 pairs (2p, 2p+1) whose q/k features live on
  partition halves 0..63 / 64..127 of the projected qkt tile.  Score
  matmuls for a pair are emitted back-to-back as K=64 row-tiled matmuls on
  disjoint row groups (tile_position (0,0) / (64,0)), so the PE array runs
  both heads CONCURRENTLY at full 128-row occupancy - 2x score throughput
  vs serial K=64 matmuls, and enough PE-cell activity to keep the HAM
  clock-gate at 8/8.
  Per (q-span, pair, k-chunk): paired score MMs -> one exp on ACT (scale
  1/8) -> triangle mask on diagonal blocks -> two PV matmuls (M=65: head
  dims + a ones column that yields the softmax denominator for free).
  Normalization: reciprocal of the denominator row on DVE, cast bf16, K=1
  bf16 matmul broadcasts 1/d across partitions (1 cyc/row vs 4 for f32),
  multiply on eviction; odd heads staged through SBUF and DMA'd to
  partitions 64..127.
  x-transposes for q-span j are emitted just before span j's work so the
  first attention block starts ~3us in; projections / output projections
  interleave into the ACT-bound attention stream via the tile scheduler.
"""
import numpy as np
from contextlib import ExitStack

import concourse.bacc as bacc
import concourse.tile as tile
from concourse import mybir
from concourse.bass_utils import run_bass_kernel_spmd

F32 = mybir.dt.float32
BF16 = mybir.dt.bfloat16
F16 = mybir.dt.float16

B = 2
S = 2048
D = 1024
HD = 64
HG = 4            # head-groups (cores per batch)
HPG = 4           # heads per group
CL = HPG * HD     # 256 local head cols per core
P = 128
NSC = S // P      # 16 s-chunks
NDC = D // P      # 8 d-chunks
NQJ = S // 512    # 4 q-spans
NKC = S // P      # 16 k-chunks

_CACHED = {}


def _build():
    if "nc" in _CACHED:
        return _CACHED["nc"]
    nc = bacc.Bacc("TRN2", target_bir_lowering=False, debug=False)

    x_d = nc.dram_tensor("x", [S, D], F16, kind="ExternalInput")
    wqk_d = nc.dram_tensor("wqk", [D, 2 * CL], F16, kind="ExternalInput")
    wv_d = nc.dram_tensor("wv", [D, CL], F16, kind="ExternalInput")
    wout_d = nc.dram_tensor("wout", [CL, D], F16, kind="ExternalInput")
    bqk_d = nc.dram_tensor("bqk", [P, 4], F32, kind="ExternalInput")
    tri_d = nc.dram_tensor("tri", [P, P], F16, kind="ExternalInput")
    idn_d = nc.dram_tensor("idn", [P, P], F16, kind="ExternalInput")
    y_d = nc.dram_tensor("y", [S, D], F32, kind="ExternalOutput")

    with tile.TileContext(nc) as tc, ExitStack() as ctx:
        persist = ctx.enter_context(tc.tile_pool(name="persist", bufs=1))
        ptp = ctx.enter_context(tc.tile_pool(name="ptp", bufs=3))
        youtp = ctx.enter_context(tc.tile_pool(name="youtp", bufs=2))
        unp = ctx.enter_context(tc.tile_pool(name="unp", bufs=2))
        rcpf = ctx.enter_context(tc.tile_pool(name="rcpf", bufs=2))
        rcpb = ctx.enter_context(tc.tile_pool(name="rcpb", bufs=2))
        tmpp = ctx.enter_context(tc.tile_pool(name="tmpp", bufs=2))
        ps_sm = ctx.enter_context(tc.tile_pool(name="ps_sm", bufs=2, space="PSUM"))
        ps_st = ctx.enter_context(tc.tile_pool(name="ps_st", bufs=2, space="PSUM"))
        ps_av = ctx.enter_context(tc.tile_pool(name="ps_av", bufs=2, space="PSUM"))

        # ---- persistent tiles ----
        qkt_sb = persist.tile([P, 4, S], F16, name="qkt_sb")        # 16KB/part
        v_sb = persist.tile([P, NKC, HPG, HD + 1], F16, name="v_sb")
        attnT = persist.tile([P, 2, S], F16, name="attnT")          # 8KB/part
        wout_sb = persist.tile([P, 2, D], F16, name="wout_sb")
        wqk_sb = persist.tile([P, NDC, 2 * CL], F16, name="wqk_sb")
        wv_sb = persist.tile([P, NDC, CL], F16, name="wv_sb")
        bqk_sb = persist.tile([P, 4], F32, name="bqk_sb")
        tri_sb = persist.tile([P, P], F16, name="tri_sb")
        ident = persist.tile([P, P], F16, name="ident")
        ones_bf = persist.tile([P, HD], BF16, name="ones_bf")
        wsrc = persist.tile([P, P], F16, name="wsrc")

        # small constants FIRST on the fast Sync HWDGE ring: ident gates the
        # warm-up matmuls and the first transposes (the SWDGE ring takes
        # ~20us to deliver, stalling the whole pipeline start)
        nc.sync.dma_start(out=ident, in_=idn_d.ap())
        nc.sync.dma_start(out=tri_sb, in_=tri_d.ap())
        nc.sync.dma_start(out=bqk_sb, in_=bqk_d.ap())
        # weights on the ACT HWDGE ring (parallel with x loads on Sync ring)
        nc.scalar.dma_start(out=wqk_sb, in_=wqk_d.ap()
                            .rearrange("(c p) m -> p c m", p=P))
        nc.scalar.dma_start(out=wv_sb, in_=wv_d.ap()
                            .rearrange("(c p) m -> p c m", p=P))
        # stationary ones row (partition 64) for the 1/d broadcast matmul
        nc.gpsimd.memset(ones_bf[64:65, :], 1.0)
        # ones column of V tiles -> free softmax denominator row in PV
        for sc in range(NSC):
            nc.gpsimd.memset(v_sb[:, sc, :, HD], 1.0)
        # wout only needed from the first output projection (~20us in)
        nc.scalar.dma_start(out=wout_sb, in_=wout_d.ap()
                            .rearrange("(c p) o -> p c o", p=P))

        with tc.tile_pool(name="xTp", bufs=1) as xTp, \
                tc.tile_pool(name="xp", bufs=2) as xp:
            xT = xTp.tile([P, NDC, S], F16, name="xT")              # 32KB/part

            # x chunk DMAs: issue all up-front, alternating HWDGE rings
            xqs = []
            for scq in range(4):
                xq = xp.tile([P, 4, D], F16, tag="x", name=f"xq{scq}")
                eng = nc.sync if scq % 2 == 0 else nc.scalar
                eng.dma_start(
                    out=xq,
                    in_=x_d.ap()[scq * 512:(scq + 1) * 512, :]
                    .rearrange("(s p) d -> p s d", p=P))
                xqs.append(xq)

            # warm the ACT table (exp) during the initial DMA wait
            warm = tmpp.tile([HD, 512], F16, tag="tmp", name="warm")
            nc.scalar.activation(warm[0:1, 0:16], tri_sb[0:1, 0:16],
                                 mybir.ActivationFunctionType.Exp, scale=1.0)

            # warm the PE HAM clock-gate during the x DMA wait: dummy
            # back-to-back matmuls on a memset tile (no DMA dependency -
            # the HWDGE rings deliver first bytes only ~10us in) flip the
            # activity monitor to 8/8 before the real pipeline starts.
            # wps lives in ps_st (first real use ~20us in) so the warm-up
            # WAW chain never gates the ps_sm rotation of the prologue.
            nc.vector.memset(wsrc, 0.5)
            wps = ps_st.tile([P, 1024], F32, tag="st", name="wps")

            def emit_warm(n):
                for w in range(n):
                    nc.tensor.matmul(wps[:, 0:P], wsrc, wsrc,
                                     start=True, stop=True)

            # ~6.4us of dummy matmuls before any DMA-dependent work: flips
            # the HAM clock-gate to 8/8 during the DMA boot window; further
            # small batches are interleaved into the span-0 projections
            # (emit_proj warm flag) to bridge its DMA-paced stalls so the
            # gate never re-throttles before the pipeline is dense.
            emit_warm(60)

            def emit_outproj(qjo):
                """Output projection for q-span qjo (y stores alternate the
                Sync and SWDGE rings so the 8MB of stores never serialize on
                one ring at the tail)."""
                for si in range(4):
                    sc = 4 * qjo + si
                    y_sb = youtp.tile([P, D], F32, tag="y", name=f"y{sc}")
                    for oc in range(2):
                        py = ps_sm.tile([P, 512], F32, tag="sm",
                                        name=f"py{sc}_{oc}")
                        for cc in range(2):
                            nc.tensor.matmul(
                                py[:],
                                attnT[:, cc, sc * P:(sc + 1) * P],
                                wout_sb[:, cc, oc * 512:(oc + 1) * 512],
                                start=(cc == 0), stop=(cc == 1))
                        nc.vector.tensor_copy(
                            y_sb[:, oc * 512:(oc + 1) * 512], py[:])
                    eng = nc.sync if sc % 2 == 0 else nc.gpsimd
                    eng.dma_start(
                        out=y_d.ap()[sc * P:(sc + 1) * P, :], in_=y_sb)

            def emit_proj(qj, warm=False):
                """Transpose x chunk qj into xT, then QK and V projection
                slices for that span (all full-util PE work)."""
                q0 = qj * 512
                xq = xqs[qj]
                for dc in range(NDC):
                    ptr = ps_sm.tile([P, 512], F16, tag="sm",
                                     name=f"ptr{qj}_{dc}")
                    for si in range(4):
                        nc.tensor.transpose(
                            ptr[:, si * P:(si + 1) * P],
                            xq[:, si, dc * P:(dc + 1) * P],
                            ident)
                    nc.vector.tensor_copy(
                        xT[:, dc, q0:q0 + 512], ptr)
                    if warm:
                        emit_warm(3)
                for mc in range(4):
                    pq = ps_sm.tile([P, 512], F32, tag="sm",
                                    name=f"pq{mc}_{qj}")
                    for kc in range(NDC):
                        nc.tensor.matmul(
                            pq[:],
                            wqk_sb[:, kc, mc * P:(mc + 1) * P],
                            xT[:, kc, q0:q0 + 512],
                            start=(kc == 0), stop=(kc == NDC - 1))
                    nc.vector.tensor_scalar_add(
                        qkt_sb[:, mc, q0:q0 + 512], pq[:],
                        bqk_sb[:, mc:mc + 1])
                    if warm:
                        emit_warm(3)
                for si in range(4):
                    sc = 4 * qj + si
                    pv = ps_sm.tile([P, CL], F32, tag="sm", name=f"pv{sc}")
                    for kc in range(NDC):
                        nc.tensor.matmul(
                            pv[:],
                            xT[:, kc, sc * P:(sc + 1) * P],
                            wv_sb[:, kc, :],
                            start=(kc == 0), stop=(kc == NDC - 1))
                    nc.vector.tensor_copy(
                        v_sb[:, sc, :, 0:HD],
                        pv.rearrange("p (h d) -> p h d", h=HPG))
                    if warm:
                        emit_warm(3)

            # prologue: span-0 projections (attention(0) needs them)
            emit_proj(0, warm=True)

            for qj in range(NQJ):
                q0 = qj * 512
                nkc = 4 * (qj + 1)

                # attention: head pairs (2p at rows 0..63, 2p+1 at 64..127)
                for p in range(2):
                    mcq, mck = p, 2 + p
                    hA, hB = 2 * p, 2 * p + 1
                    avA = ps_av.tile([P, 512], F32, tag="av",
                                     name=f"avA{qj}_{p}")
                    avB = ps_av.tile([P, 512], F32, tag="av",
                                     name=f"avB{qj}_{p}")
                    for kc in range(nkc):
                        t = kc - 4 * qj
                        c0 = 128 * t if t > 0 else 0
                        stp = ps_st.tile([P, 1024], F32, tag="st",
                                         name=f"st{qj}_{p}_{kc}")
                        # paired K=64 score matmuls on disjoint row groups:
                        # run concurrently on the PE array
                        nc.tensor.matmul(
                            stp[:, c0:512],
                            qkt_sb[0:HD, mck, kc * P:(kc + 1) * P],
                            qkt_sb[0:HD, mcq, q0 + c0:q0 + 512],
                            start=True, stop=True)
                        nc.tensor.matmul(
                            stp[:, 512 + c0:1024],
                            qkt_sb[HD:P, mck, kc * P:(kc + 1) * P],
                            qkt_sb[HD:P, mcq, q0 + c0:q0 + 512],
                            start=True, stop=True)
                        pt = ptp.tile([P, 1024], F16, tag="pt",
                                      name=f"pt{qj}_{p}_{kc}")
                        nc.scalar.activation(
                            pt[:, c0:1024], stp[:, c0:1024],
                            mybir.ActivationFunctionType.Exp, scale=0.125)
                        if 0 <= t <= 3:
                            nc.vector.tensor_mul(
                                pt[:, c0:c0 + 128],
                                pt[:, c0:c0 + 128], tri_sb)
                            nc.vector.tensor_mul(
                                pt[:, 512 + c0:512 + c0 + 128],
                                pt[:, 512 + c0:512 + c0 + 128], tri_sb)
                        nc.tensor.matmul(
                            avA[0:HD + 1, c0:512],
                            v_sb[:, kc, hA, :],
                            pt[:, c0:512],
                            start=(kc == 0), stop=(kc == nkc - 1))
                        nc.tensor.matmul(
                            avB[0:HD + 1, c0:512],
                            v_sb[:, kc, hB, :],
                            pt[:, 512 + c0:1024],
                            start=(kc == 0), stop=(kc == nkc - 1))

                    # normalize: evict both heads first (frees both av
                    # slots), then per head: bf16 denominator broadcast via
                    # K=1 matmul (dnb reuses the freed av slots, keeping
                    # ps_sm free of attention-phase allocations), reciprocal,
                    # multiply. Odd head first so its partition-shift DMA
                    # hides behind the even head's chain.
                    unB = unp.tile([HD + 1, 512], F32, tag="un",
                                   name=f"unB{qj}_{p}")
                    nc.vector.tensor_copy(unB, avB[0:HD + 1, :])
                    unA = unp.tile([HD + 1, 512], F32, tag="un",
                                   name=f"unA{qj}_{p}")
                    nc.vector.tensor_copy(unA, avA[0:HD + 1, :])
                    for un, h in ((unB, hB), (unA, hA)):
                        rcb = rcpb.tile([HD + 1, 512], BF16, tag="rcb",
                                        name=f"rcb{qj}_{h}")
                        nc.vector.tensor_copy(rcb[HD:HD + 1, :],
                                              un[HD:HD + 1, :])
                        dnb = ps_av.tile([P, 512], F32, tag="av",
                                         name=f"dnb{qj}_{h}")
                        nc.tensor.matmul(dnb[0:HD, :],
                                         ones_bf[HD:HD + 1, :],
                                         rcb[HD:HD + 1, :],
                                         start=True, stop=True)
                        rbs = rcpf.tile([HD, 512], F32, tag="rcf",
                                        name=f"rbs{qj}_{h}")
                        nc.vector.reciprocal_approx_fast(rbs, dnb[0:HD, :])
                        if h % 2 == 0:
                            nc.vector.tensor_mul(
                                attnT[0:HD, p, q0:q0 + 512],
                                un[0:HD, :], rbs)
                        else:
                            tmp = tmpp.tile([HD, 512], F16, tag="tmp",
                                            name=f"tmp{qj}_{h}")
                            nc.vector.tensor_mul(tmp, un[0:HD, :], rbs)
                            nc.sync.dma_start(
                                out=attnT[HD:P, p, q0:q0 + 512], in_=tmp)

                # fill work emitted AFTER this span's attention, so the
                # scheduler (priority = program order) runs it only when
                # attention matmuls are stalled on the exp stream: next
                # span's projections; in the last span - where no proj work
                # remains - all four output projections.
                if qj < NQJ - 1:
                    emit_proj(qj + 1)
                else:
                    for qjo in range(NQJ):
                        emit_outproj(qjo)

    nc.compile()
    _CACHED["nc"] = nc
    return nc


def _host_inputs(x, W_qkv, b_qkv):
    """Build the 8 per-core input maps (wout filled in by caller)."""
    x16 = np.asarray(x, dtype=np.float16)
    tri = (np.arange(P)[None, :] >= np.arange(P)[:, None]).astype(np.float16)
    in_maps = []
    for b in range(B):
        for hg in range(HG):
            c0 = hg * CL
            wqk = np.ascontiguousarray(
                np.concatenate([W_qkv[:, c0:c0 + CL],
                                W_qkv[:, D + c0:D + c0 + CL]], axis=1)
                .astype(np.float16))
            wv = np.ascontiguousarray(
                W_qkv[:, 2 * D + c0:2 * D + c0 + CL].astype(np.float16))
            bqk = np.ascontiguousarray(
                np.concatenate([b_qkv[c0:c0 + CL],
                                b_qkv[D + c0:D + c0 + CL]])
                .reshape(4, P).T, dtype=np.float32)
            in_maps.append({
                "x": x16[b], "wqk": wqk, "wv": wv, "wout": None,
                "bqk": bqk, "tri": tri, "idn": np.eye(P, dtype=np.float16),
            })
    return in_maps


def kernel(x, W_qkv, b_qkv, W_out, b_out):
    x = np.asarray(x, dtype=np.float32)
    W_qkv = np.asarray(W_qkv, dtype=np.float32)
    b_qkv = np.asarray(b_qkv, dtype=np.float32)
    W_out = np.asarray(W_out, dtype=np.float32)
    b_out = np.asarray(b_out, dtype=np.float32)

    nc = _build()
    in_maps = _host_inputs(x, W_qkv, b_qkv)
    for i, m in enumerate(in_maps):
        hg = i % HG
        m["wout"] = np.ascontiguousarray(
            W_out[hg * CL:(hg + 1) * CL, :].astype(np.float16))
    core_ids = list(range(8))
    res = run_bass_kernel_spmd(nc, in_maps, core_ids)
    outs = [r["y"] for r in res.results]
    bv = b_qkv[2 * D:3 * D]
    corr = (bv @ W_out + b_out).astype(np.float32)
    y = np.empty((B, S, D), dtype=np.float32)
    for b in range(B):
        acc = outs[b * HG].astype(np.float32).copy()
        for hg in range(1, HG):
            acc += outs[b * HG + hg]
        y[b] = acc + corr
    return y


# revision 33
# speedup vs baseline: 1.0695x; 1.0176x over previous
"""Causal self-attention Trainium2 kernel.

Reference (full): x[B=2,S=2048,D=1024] @ W_qkv + b_qkv -> 16-head causal
attention -> @ W_out + b_out.

Sharding: 8 cores = (batch b in 0..1) x (head-group hg in 0..3, 4 heads of
hd=64 each). Each core computes a partial output projection for its 4 heads
on its batch; the host sums the 4 head-group partials per batch and adds the
(constant) V-bias correction bv @ W_out and b_out.

Device pipeline per core (data path fp16, accumulation fp32 PSUM):
  The 4 local heads form 2 pairs (2p, 2p+1) whose q/k features live on
  partition halves 0..63 / 64..127 of the projected qkt tile.  Score
  matmuls for a pair are emitted back-to-back as K=64 row-tiled matmuls on
  disjoint row groups (tile_position (0,0) / (64,0)), so the PE array runs
  both heads CONCURRENTLY at full 128-row occupancy - 2x score throughput
  vs serial K=64 matmuls, and enough PE-cell activity to keep the HAM
  clock-gate at 8/8.
  Per (q-span, pair, k-chunk): paired score MMs -> one exp on ACT (scale
  1/8) -> triangle mask on diagonal blocks -> two PV matmuls (M=65: head
  dims + a ones column that yields the softmax denominator for free).
  Normalization: reciprocal of the denominator row on DVE, cast bf16, K=1
  bf16 matmul broadcasts 1/d across partitions (1 cyc/row vs 4 for f32),
  multiply on eviction; odd heads staged through SBUF and DMA'd to
  partitions 64..127.
  x-transposes for q-span j are emitted just before span j's work so the
  first attention block starts ~3us in; projections / output projections
  interleave into the ACT-bound attention stream via the tile scheduler.
"""
import numpy as np
from contextlib import ExitStack

import concourse.bacc as bacc
import concourse.tile as tile
from concourse import mybir
from concourse.bass_utils import run_bass_kernel_spmd

F32 = mybir.dt.float32
BF16 = mybir.dt.bfloat16
F16 = mybir.dt.float16

B = 2
S = 2048
D = 1024
HD = 64
HG = 4            # head-groups (cores per batch)
HPG = 4           # heads per group
CL = HPG * HD     # 256 local head cols per core
P = 128
NSC = S // P      # 16 s-chunks
NDC = D // P      # 8 d-chunks
NQJ = S // 512    # 4 q-spans
NKC = S // P      # 16 k-chunks

_CACHED = {}


def _build():
    if "nc" in _CACHED:
        return _CACHED["nc"]
    nc = bacc.Bacc("TRN2", target_bir_lowering=False, debug=False)

    x_d = nc.dram_tensor("x", [S, D], F16, kind="ExternalInput")
    wqk_d = nc.dram_tensor("wqk", [D, 2 * CL], F16, kind="ExternalInput")
    wv_d = nc.dram_tensor("wv", [D, CL], F16, kind="ExternalInput")
    wout_d = nc.dram_tensor("wout", [CL, D], F16, kind="ExternalInput")
    bqk_d = nc.dram_tensor("bqk", [P, 4], F32, kind="ExternalInput")
    tri_d = nc.dram_tensor("tri", [P, P], F16, kind="ExternalInput")
    idn_d = nc.dram_tensor("idn", [P, P], F16, kind="ExternalInput")
    y_d = nc.dram_tensor("y", [S, D], F32, kind="ExternalOutput")

    with tile.TileContext(nc) as tc, ExitStack() as ctx:
        persist = ctx.enter_context(tc.tile_pool(name="persist", bufs=1))
        ptp = ctx.enter_context(tc.tile_pool(name="ptp", bufs=3))
        youtp = ctx.enter_context(tc.tile_pool(name="youtp", bufs=3))
        unp = ctx.enter_context(tc.tile_pool(name="unp", bufs=2))
        rcpf = ctx.enter_context(tc.tile_pool(name="rcpf", bufs=2))
        rcpb = ctx.enter_context(tc.tile_pool(name="rcpb", bufs=2))
        tmpp = ctx.enter_context(tc.tile_pool(name="tmpp", bufs=2))
        ps_sm = ctx.enter_context(tc.tile_pool(name="ps_sm", bufs=2, space="PSUM"))
        ps_st = ctx.enter_context(tc.tile_pool(name="ps_st", bufs=2, space="PSUM"))
        ps_av = ctx.enter_context(tc.tile_pool(name="ps_av", bufs=2, space="PSUM"))

        # ---- persistent tiles ----
        qkt_sb = persist.tile([P, 4, S], F16, name="qkt_sb")        # 16KB/part
        v_sb = persist.tile([P, NKC, HPG, HD + 1], F16, name="v_sb")
        attnT = persist.tile([P, 2, S], F16, name="attnT")          # 8KB/part
        wout_sb = persist.tile([P, 2, D], F16, name="wout_sb")
        wqk_sb = persist.tile([P, NDC, 2 * CL], F16, name="wqk_sb")
        wv_sb = persist.tile([P, NDC, CL], F16, name="wv_sb")
        bqk_sb = persist.tile([P, 4], F32, name="bqk_sb")
        tri_sb = persist.tile([P, P], F16, name="tri_sb")
        ident = persist.tile([P, P], F16, name="ident")
        ones_bf = persist.tile([P, HD], BF16, name="ones_bf")
        wsrc = persist.tile([P, P], F16, name="wsrc")

        # small constants FIRST on the fast Sync HWDGE ring: ident gates the
        # warm-up matmuls and the first transposes (the SWDGE ring takes
        # ~20us to deliver, stalling the whole pipeline start)
        nc.sync.dma_start(out=ident, in_=idn_d.ap())
        nc.sync.dma_start(out=tri_sb, in_=tri_d.ap())
        nc.sync.dma_start(out=bqk_sb, in_=bqk_d.ap())
        # weights on the ACT HWDGE ring (parallel with x loads on Sync ring)
        nc.scalar.dma_start(out=wqk_sb, in_=wqk_d.ap()
                            .rearrange("(c p) m -> p c m", p=P))
        nc.scalar.dma_start(out=wv_sb, in_=wv_d.ap()
                            .rearrange("(c p) m -> p c m", p=P))
        # stationary ones row (partition 64) for the 1/d broadcast matmul
        nc.gpsimd.memset(ones_bf[64:65, :], 1.0)
        # ones column of V tiles -> free softmax denominator row in PV
        for sc in range(NSC):
            nc.gpsimd.memset(v_sb[:, sc, :, HD], 1.0)
        # wout only needed from the first output projection (~20us in)
        nc.scalar.dma_start(out=wout_sb, in_=wout_d.ap()
                            .rearrange("(c p) o -> p c o", p=P))

        with tc.tile_pool(name="xTp", bufs=1) as xTp, \
                tc.tile_pool(name="xp", bufs=2) as xp:
            xT = xTp.tile([P, NDC, S], F16, name="xT")              # 32KB/part

            # x chunk DMAs: issue all up-front, alternating HWDGE rings
            xqs = []
            for scq in range(4):
                xq = xp.tile([P, 4, D], F16, tag="x", name=f"xq{scq}")
                eng = nc.sync if scq % 2 == 0 else nc.scalar
                eng.dma_start(
                    out=xq,
                    in_=x_d.ap()[scq * 512:(scq + 1) * 512, :]
                    .rearrange("(s p) d -> p s d", p=P))
                xqs.append(xq)

            # warm the ACT table (exp) during the initial DMA wait
            warm = tmpp.tile([HD, 512], F16, tag="tmp", name="warm")
            nc.scalar.activation(warm[0:1, 0:16], tri_sb[0:1, 0:16],
                                 mybir.ActivationFunctionType.Exp, scale=1.0)

            # warm the PE HAM clock-gate during the x DMA wait: dummy
            # back-to-back matmuls on a memset tile (no DMA dependency -
            # the HWDGE rings deliver first bytes only ~10us in) flip the
            # activity monitor to 8/8 before the real pipeline starts.
            # wps lives in ps_st (first real use ~20us in) so the warm-up
            # WAW chain never gates the ps_sm rotation of the prologue.
            nc.vector.memset(wsrc, 0.5)
            wps = ps_st.tile([P, 1024], F32, tag="st", name="wps")

            def emit_warm(n):
                for w in range(n):
                    nc.tensor.matmul(wps[:, 0:P], wsrc, wsrc,
                                     start=True, stop=True)

            # ~6.4us of dummy matmuls before any DMA-dependent work: flips
            # the HAM clock-gate to 8/8 during the DMA boot window; further
            # small batches are interleaved into the span-0 projections
            # (emit_proj warm flag) to bridge its DMA-paced stalls so the
            # gate never re-throttles before the pipeline is dense.
            emit_warm(60)

            def emit_outproj(qjo):
                """Output projection for q-span qjo (y stores alternate the
                Sync and SWDGE rings so the 8MB of stores never serialize on
                one ring at the tail). For the LAST span - the kernel tail,
                where the exp stream is done - evictions alternate onto the
                then-idle Scalar engine to halve the eviction chain."""
                for si in range(4):
                    sc = 4 * qjo + si
                    y_sb = youtp.tile([P, D], F32, tag="y", name=f"y{sc}")
                    for oc in range(2):
                        py = ps_sm.tile([P, 512], F32, tag="sm",
                                        name=f"py{sc}_{oc}")
                        for cc in range(2):
                            nc.tensor.matmul(
                                py[:],
                                attnT[:, cc, sc * P:(sc + 1) * P],
                                wout_sb[:, cc, oc * 512:(oc + 1) * 512],
                                start=(cc == 0), stop=(cc == 1))
                        if qjo == NQJ - 1 and oc == 1:
                            nc.scalar.copy(
                                y_sb[:, oc * 512:(oc + 1) * 512], py[:])
                        else:
                            nc.vector.tensor_copy(
                                y_sb[:, oc * 512:(oc + 1) * 512], py[:])
                    eng = nc.sync if sc % 2 == 0 else nc.gpsimd
                    eng.dma_start(
                        out=y_d.ap()[sc * P:(sc + 1) * P, :], in_=y_sb)

            def emit_proj(qj, warm=False):
                """Transpose x chunk qj into xT, then QK and V projection
                slices for that span (all full-util PE work)."""
                q0 = qj * 512
                xq = xqs[qj]
                for dc in range(NDC):
                    ptr = ps_sm.tile([P, 512], F16, tag="sm",
                                     name=f"ptr{qj}_{dc}")
                    for si in range(4):
                        nc.tensor.transpose(
                            ptr[:, si * P:(si + 1) * P],
                            xq[:, si, dc * P:(dc + 1) * P],
                            ident)
                    nc.vector.tensor_copy(
                        xT[:, dc, q0:q0 + 512], ptr)
                    if warm:
                        emit_warm(3)
                for mc in range(4):
                    pq = ps_sm.tile([P, 512], F32, tag="sm",
                                    name=f"pq{mc}_{qj}")
                    for kc in range(NDC):
                        nc.tensor.matmul(
                            pq[:],
                            wqk_sb[:, kc, mc * P:(mc + 1) * P],
                            xT[:, kc, q0:q0 + 512],
                            start=(kc == 0), stop=(kc == NDC - 1))
                    nc.vector.tensor_scalar_add(
                        qkt_sb[:, mc, q0:q0 + 512], pq[:],
                        bqk_sb[:, mc:mc + 1])
                    if warm:
                        emit_warm(3)
                for si in range(4):
                    sc = 4 * qj + si
                    pv = ps_sm.tile([P, CL], F32, tag="sm", name=f"pv{sc}")
                    for kc in range(NDC):
                        nc.tensor.matmul(
                            pv[:],
                            xT[:, kc, sc * P:(sc + 1) * P],
                            wv_sb[:, kc, :],
                            start=(kc == 0), stop=(kc == NDC - 1))
                    nc.vector.tensor_copy(
                        v_sb[:, sc, :, 0:HD],
                        pv.rearrange("p (h d) -> p h d", h=HPG))
                    if warm:
                        emit_warm(3)

            def emit_proj0():
                """Span-0 projections restructured for the DMA-racing
                startup: per d-chunk, the transposes are immediately
                followed by the projection matmuls consuming that chunk
                (dc-outer accumulation). Transposes are invisible to the
                HAM activity monitor, so interleaving real matmuls keeps
                the clock-gate at 8/8 from ~14us on. The q/k feature
                accumulators live as column halves of ps_st slots and the
                V accumulators as halves of a ps_av slot - ps_sm keeps
                only the transpose staging tiles.
                Pass 0 computes mc 0,1 (q features) + s-chunks 0,1; pass 1
                (xT resident) computes mc 2,3 (k features) + s-chunks 2,3."""
                for half in range(2):
                    # pqt's two 512-col accumulators land in separate PSUM
                    # banks; the V accumulators need separate TILES - a
                    # matmul's start=True clears its whole bank, so two
                    # accumulation groups must not share one.
                    pqt = ps_st.tile([P, 1024], F32, tag="st",
                                     name=f"pq0h{half}")
                    pvts = [ps_av.tile([P, 512], F32, tag="av",
                                       name=f"pv0h{half}_{j}")
                            for j in range(2)]
                    for dc in range(NDC):
                        if half == 0:
                            ptr = ps_sm.tile([P, 512], F16, tag="sm",
                                             name=f"ptr0_{dc}")
                            for si in range(4):
                                nc.tensor.transpose(
                                    ptr[:, si * P:(si + 1) * P],
                                    xqs[0][:, si, dc * P:(dc + 1) * P],
                                    ident)
                            nc.vector.tensor_copy(xT[:, dc, 0:512], ptr)
                        for j in range(2):
                            mc = 2 * half + j
                            nc.tensor.matmul(
                                pqt[:, j * 512:(j + 1) * 512],
                                wqk_sb[:, dc, mc * P:(mc + 1) * P],
                                xT[:, dc, 0:512],
                                start=(dc == 0), stop=(dc == NDC - 1))
                        for j in range(2):
                            sc = 2 * half + j
                            nc.tensor.matmul(
                                pvts[j][:, 0:CL],
                                xT[:, dc, sc * P:(sc + 1) * P],
                                wv_sb[:, dc, :],
                                start=(dc == 0), stop=(dc == NDC - 1))
                    for j in range(2):
                        mc = 2 * half + j
                        nc.vector.tensor_scalar_add(
                            qkt_sb[:, mc, 0:512],
                            pqt[:, j * 512:(j + 1) * 512],
                            bqk_sb[:, mc:mc + 1])
                    for j in range(2):
                        sc = 2 * half + j
                        nc.vector.tensor_copy(
                            v_sb[:, sc, :, 0:HD],
                            pvts[j][:, 0:CL]
                            .rearrange("p (h d) -> p h d", h=HPG))

            # prologue: span-0 projections (attention(0) needs them)
            emit_proj0()

            for qj in range(NQJ):
                q0 = qj * 512
                nkc = 4 * (qj + 1)

                # attention: head pairs (2p at rows 0..63, 2p+1 at 64..127)
                for p in range(2):
                    mcq, mck = p, 2 + p
                    hA, hB = 2 * p, 2 * p + 1
                    avA = ps_av.tile([P, 512], F32, tag="av",
                                     name=f"avA{qj}_{p}")
                    avB = ps_av.tile([P, 512], F32, tag="av",
                                     name=f"avB{qj}_{p}")
                    for kc in range(nkc):
                        t = kc - 4 * qj
                        c0 = 128 * t if t > 0 else 0
                        stp = ps_st.tile([P, 1024], F32, tag="st",
                                         name=f"st{qj}_{p}_{kc}")
                        # paired K=64 score matmuls on disjoint row groups:
                        # run concurrently on the PE array
                        nc.tensor.matmul(
                            stp[:, c0:512],
                            qkt_sb[0:HD, mck, kc * P:(kc + 1) * P],
                            qkt_sb[0:HD, mcq, q0 + c0:q0 + 512],
                            start=True, stop=True)
                        nc.tensor.matmul(
                            stp[:, 512 + c0:1024],
                            qkt_sb[HD:P, mck, kc * P:(kc + 1) * P],
                            qkt_sb[HD:P, mcq, q0 + c0:q0 + 512],
                            start=True, stop=True)
                        pt = ptp.tile([P, 1024], F16, tag="pt",
                                      name=f"pt{qj}_{p}_{kc}")
                        nc.scalar.activation(
                            pt[:, c0:1024], stp[:, c0:1024],
                            mybir.ActivationFunctionType.Exp, scale=0.125)
                        if 0 <= t <= 3:
                            nc.vector.tensor_mul(
                                pt[:, c0:c0 + 128],
                                pt[:, c0:c0 + 128], tri_sb)
                            nc.vector.tensor_mul(
                                pt[:, 512 + c0:512 + c0 + 128],
                                pt[:, 512 + c0:512 + c0 + 128], tri_sb)
                        nc.tensor.matmul(
                            avA[0:HD + 1, c0:512],
                            v_sb[:, kc, hA, :],
                            pt[:, c0:512],
                            start=(kc == 0), stop=(kc == nkc - 1))
                        nc.tensor.matmul(
                            avB[0:HD + 1, c0:512],
                            v_sb[:, kc, hB, :],
                            pt[:, 512 + c0:1024],
                            start=(kc == 0), stop=(kc == nkc - 1))

                    # normalize: evict both heads first (frees both av
                    # slots), then per head: bf16 denominator broadcast via
                    # K=1 matmul (dnb reuses the freed av slots, keeping
                    # ps_sm free of attention-phase allocations), reciprocal,
                    # multiply. Odd head first so its partition-shift DMA
                    # hides behind the even head's chain.
                    unB = unp.tile([HD + 1, 512], F32, tag="un",
                                   name=f"unB{qj}_{p}")
                    nc.vector.tensor_copy(unB, avB[0:HD + 1, :])
                    unA = unp.tile([HD + 1, 512], F32, tag="un",
                                   name=f"unA{qj}_{p}")
                    nc.vector.tensor_copy(unA, avA[0:HD + 1, :])
                    for un, h in ((unB, hB), (unA, hA)):
                        rcb = rcpb.tile([HD + 1, 512], BF16, tag="rcb",
                                        name=f"rcb{qj}_{h}")
                        nc.vector.tensor_copy(rcb[HD:HD + 1, :],
                                              un[HD:HD + 1, :])
                        dnb = ps_av.tile([P, 512], F32, tag="av",
                                         name=f"dnb{qj}_{h}")
                        nc.tensor.matmul(dnb[0:HD, :],
                                         ones_bf[HD:HD + 1, :],
                                         rcb[HD:HD + 1, :],
                                         start=True, stop=True)
                        rbs = rcpf.tile([HD, 512], F32, tag="rcf",
                                        name=f"rbs{qj}_{h}")
                        nc.vector.reciprocal_approx_fast(rbs, dnb[0:HD, :])
                        if h % 2 == 0:
                            nc.vector.tensor_mul(
                                attnT[0:HD, p, q0:q0 + 512],
                                un[0:HD, :], rbs)
                        else:
                            tmp = tmpp.tile([HD, 512], F16, tag="tmp",
                                            name=f"tmp{qj}_{h}")
                            nc.vector.tensor_mul(tmp, un[0:HD, :], rbs)
                            nc.sync.dma_start(
                                out=attnT[HD:P, p, q0:q0 + 512], in_=tmp)

                # fill work emitted AFTER this span's attention, so the
                # scheduler (priority = program order) runs it only when
                # attention matmuls are stalled on the exp stream: next
                # span's projections; in the last span - where no proj work
                # remains - all four output projections.
                if qj < NQJ - 1:
                    emit_proj(qj + 1)
                else:
                    for qjo in range(NQJ):
                        emit_outproj(qjo)

    nc.compile()
    _CACHED["nc"] = nc
    return nc


def _host_inputs(x, W_qkv, b_qkv):
    """Build the 8 per-core input maps (wout filled in by caller)."""
    x16 = np.asarray(x, dtype=np.float16)
    tri = (np.arange(P)[None, :] >= np.arange(P)[:, None]).astype(np.float16)
    in_maps = []
    for b in range(B):
        for hg in range(HG):
            c0 = hg * CL
            wqk = np.ascontiguousarray(
                np.concatenate([W_qkv[:, c0:c0 + CL],
                                W_qkv[:, D + c0:D + c0 + CL]], axis=1)
                .astype(np.float16))
            wv = np.ascontiguousarray(
                W_qkv[:, 2 * D + c0:2 * D + c0 + CL].astype(np.float16))
            bqk = np.ascontiguousarray(
                np.concatenate([b_qkv[c0:c0 + CL],
                                b_qkv[D + c0:D + c0 + CL]])
                .reshape(4, P).T, dtype=np.float32)
            in_maps.append({
                "x": x16[b], "wqk": wqk, "wv": wv, "wout": None,
                "bqk": bqk, "tri": tri, "idn": np.eye(P, dtype=np.float16),
            })
    return in_maps


def kernel(x, W_qkv, b_qkv, W_out, b_out):
    x = np.asarray(x, dtype=np.float32)
    W_qkv = np.asarray(W_qkv, dtype=np.float32)
    b_qkv = np.asarray(b_qkv, dtype=np.float32)
    W_out = np.asarray(W_out, dtype=np.float32)
    b_out = np.asarray(b_out, dtype=np.float32)

    nc = _build()
    in_maps = _host_inputs(x, W_qkv, b_qkv)
    for i, m in enumerate(in_maps):
        hg = i % HG
        m["wout"] = np.ascontiguousarray(
            W_out[hg * CL:(hg + 1) * CL, :].astype(np.float16))
    core_ids = list(range(8))
    res = run_bass_kernel_spmd(nc, in_maps, core_ids)
    outs = [r["y"] for r in res.results]
    bv = b_qkv[2 * D:3 * D]
    corr = (bv @ W_out + b_out).astype(np.float32)
    y = np.empty((B, S, D), dtype=np.float32)
    for b in range(B):
        acc = outs[b * HG].astype(np.float32).copy()
        for hg in range(1, HG):
            acc += outs[b * HG + hg]
        y[b] = acc + corr
    return y
